# revision 1
# baseline (speedup 1.0000x reference)
"""Masked-BCE valid-region loss on 8 Trainium2 NeuronCores.

Inputs (full): cancer_logits [32,1,512,512] f32, label [32] f32,
prostate_mask [32,1,512,512] f32, needle_mask [32,1,512,512] f32.
Output: scalar f32 loss.

Sharding: data-parallel over batch — 4 images per core. Host packs the
three big tensors into one [IMG, 128, 3, 2048] input per core so each
image is a single 3MB DMA. Per image the device computes partial sums
(sum of masked logits per image via a fused scalar_tensor_tensor, sum of
softplus(masked logits) per image via the ACT accumulator, and a global
mask count via a TensorE ones-matmul reduction); the host combines them:

    bce = softplus(x) - x*y            (y constant per image)
    sum(bce*m) = sum_masked softplus(x) - y * sum(x*m)
    softplus(x*m) = softplus(x) where m==1, ln(2) where m==0
 => sum_masked softplus(x) = sum softplus(x*m) - (N - count)*ln(2)

so no label and no mask-gated softplus multiply is needed on device.
"""

import sys

for _p in ("/opt/trn_rl_repo", "/root/.axon_site/_ro/trn_rl_repo"):
    if _p not in sys.path:
        sys.path.append(_p)

import numpy as np

import concourse.bacc as bacc
import concourse.bass as bass
import concourse.tile as tile
from concourse import mybir
from concourse.bass_utils import run_bass_kernel_spmd

B, H, W = 32, 512, 512
N_CORES = 8
IMGS_PER_CORE = B // N_CORES  # 4
P = 128
FD = (H * W) // P  # 2048 free-dim elements per partition per image
N_PER_IMG = H * W  # 262144

_nc_cache = None


def _patch_act_tables():
    """Steer every activation to `natural_log_exp_and_others` (it holds
    exp, ln, sign, copy, identity) by blanking the other sets' function
    lists. The per-activation table picker takes the first set containing
    the function, so without this Exp->set0 / Ln->natural_log alternate
    and bacc emits a ~1.3us ACT_TABLE_LOAD before nearly every ACTIVATE.
    Set positions are preserved, so the emitted act_func_set_id still
    matches act_info.json and the correct table is loaded."""
    import concourse.hw_specs as hw_specs

    if getattr(bacc, "_act_tables_patched", False):
        return
    orig = hw_specs.get_activation_tables

    def patched(module_arch):
        tables = orig(module_arch)
        keep = "natural_log_exp_and_others"
        if keep in tables:
            tables = {
                name: (funcs if name == keep else set())
                for name, funcs in tables.items()
            }
        return tables

    bacc.get_activation_tables = patched
    bacc._act_tables_patched = True


def _build_bass():
    _patch_act_tables()
    f32 = mybir.dt.float32
    # Bacc (not plain Bass): its finalize() runs generate_event_semaphores,
    # which splits multi-semaphore sync waits into single-wait EventSemaphore
    # instructions — walrus codegen rejects instructions with >1 sync wait.
    nc = bacc.Bacc()
    # Per image: [128 partitions, {logits, prostate, needle}, 2048].
    xpn_d = nc.dram_tensor("xpn", [IMGS_PER_CORE, P, 3, FD], f32, kind="ExternalInput")
    # stats[:,0] = per-chunk sum(x*m), stats[:,1] = per-chunk
    # sum(softplus(x*m)); both per-partition. The last image is processed
    # as two half-image chunks (shorter serial tail after its DMA lands),
    # so there are IMGS_PER_CORE+1 chunk columns; host folds the last two.
    n_cols = IMGS_PER_CORE + 1
    out_d = nc.dram_tensor("stats", [P, 2, n_cols], f32, kind="ExternalOutput")
    # Mask count, reduced over partitions by TensorE; host sums the 512.
    cnt_d = nc.dram_tensor("cnt", [1, 512], f32, kind="ExternalOutput")

    with tile.TileContext(nc) as tc:
        with (
            tc.tile_pool(name="io", bufs=1) as io_pool,
            tc.tile_pool(name="xm", bufs=4) as xm_pool,
            tc.tile_pool(name="work", bufs=4) as work_pool,
            tc.tile_pool(name="stats", bufs=1) as stats_pool,
            tc.tile_pool(name="psum", bufs=1, space="PSUM") as psum_pool,
        ):
            dve_stats = stats_pool.tile([P, 1, n_cols], f32)
            out_stats = stats_pool.tile([P, 2, n_cols], f32)
            sxm = dve_stats[:, 0, :]
            ssp = out_stats[:, 1, :]
            ones = stats_pool.tile([P, 1], f32)
            nc.vector.memset(ones, 1.0)
            cnt_ps = psum_pool.tile([1, 512], f32)

            # chunks: (pn tile, x tile, stats column). Each image's load is
            # two DMAs — [p,n] first, then [x] (FIFO order) — so the
            # min/mask DVE work overlaps the logits transfer; the last
            # image is also split into halves to shorten the serial tail.
            HF = FD // 2
            last = IMGS_PER_CORE - 1
            chunk_tiles = []
            for i in range(last):
                tpn = io_pool.tile([P, 2, FD], f32, tag=f"pn{i}")
                tx = io_pool.tile([P, FD], f32, tag=f"x{i}")
                nc.sync.dma_start(out=tpn, in_=xpn_d[i][:, 1:3, :])
                nc.sync.dma_start(out=tx, in_=xpn_d[i][:, 0, :])
                chunk_tiles.append((tpn, tx, i))
            for h in range(2):
                sl = slice(h * HF, (h + 1) * HF)
                tpn = io_pool.tile([P, 2, HF], f32, tag=f"pn{last}h{h}")
                tx = io_pool.tile([P, HF], f32, tag=f"x{last}h{h}")
                nc.sync.dma_start(out=tpn, in_=xpn_d[last][:, 1:3, sl])
                nc.sync.dma_start(out=tx, in_=xpn_d[last][:, 0, sl])
                chunk_tiles.append((tpn, tx, last + h))

            total_mms = sum(t.shape[2] // 512 for t, _, _ in chunk_tiles)
            mm_done = 0
            for tpn, tx, i in chunk_tiles:
                cfd = tpn.shape[2]
                pt = tpn[:, 0, :]
                nt = tpn[:, 1, :]

                # pt = min(p, n); (min > 0.5) == (p > 0.5) & (n > 0.5).
                nc.vector.tensor_tensor(
                    out=pt, in0=pt, in1=nt, op=mybir.AluOpType.min
                )
                # xm = (min > 0.5) * x with fused per-partition sum(xm).
                # Emitted before the mask pass so ACT can start soonest.
                xmt = xm_pool.tile([P, cfd], f32, tag="xmt")
                nc.vector.scalar_tensor_tensor(
                    out=xmt,
                    in0=pt,
                    scalar=0.5,
                    in1=tx,
                    op0=mybir.AluOpType.is_gt,
                    op1=mybir.AluOpType.mult,
                    accum_out=sxm[:, i : i + 1],
                )
                # mask m = (pt > 0.5) in {0.0, 1.0} for the TensorE count.
                nc.vector.tensor_scalar(
                    out=nt,
                    in0=pt,
                    scalar1=0.5,
                    scalar2=None,
                    op0=mybir.AluOpType.is_gt,
                )
                # count: TensorE reduces m over partitions into PSUM.
                n_sub = cfd // 512
                for c in range(n_sub):
                    nc.tensor.matmul(
                        cnt_ps,
                        ones,
                        nt[:, c * 512 : (c + 1) * 512],
                        start=(mm_done == 0),
                        stop=(mm_done == total_mms - 1),
                    )
                    mm_done += 1
                # softplus(xm) = ln(exp(xm) + 1); |xm| <= ~6 so exp is safe.
                et = work_pool.tile([P, cfd], f32, tag="et")
                nc.scalar.activation(
                    out=et, in_=xmt, func=mybir.ActivationFunctionType.Exp
                )
                nc.scalar.activation(
                    out=et,
                    in_=et,
                    func=mybir.ActivationFunctionType.Ln,
                    bias=1.0,
                    accum_out=ssp[:, i : i + 1],
                )

            # Move DVE/PE-written results into ACT-owned tiles so each output
            # DMA waits on the ACT semaphore only.
            nc.scalar.activation(
                out=out_stats[:, 0:1, :],
                in_=dve_stats,
                func=mybir.ActivationFunctionType.Copy,
            )
            cnt_sb = stats_pool.tile([1, 512], f32)
            nc.scalar.activation(
                out=cnt_sb, in_=cnt_ps, func=mybir.ActivationFunctionType.Copy
            )
            nc.sync.dma_start(out=out_d[:], in_=out_stats)
            nc.sync.dma_start(out=cnt_d[:], in_=cnt_sb)
    nc.finalize()
    return nc


def _get_nc():
    global _nc_cache
    if _nc_cache is None:
        _nc_cache = _build_bass()
    return _nc_cache


def _make_in_maps(cancer_logits, prostate_mask, needle_mask):
    x = np.ascontiguousarray(cancer_logits, dtype=np.float32).reshape(B, P, FD)
    p = np.ascontiguousarray(prostate_mask, dtype=np.float32).reshape(B, P, FD)
    n = np.ascontiguousarray(needle_mask, dtype=np.float32).reshape(B, P, FD)
    xpn = np.empty((B, P, 3, FD), dtype=np.float32)
    xpn[:, :, 0, :] = x
    xpn[:, :, 1, :] = p
    xpn[:, :, 2, :] = n
    return [
        {"xpn": xpn[c * IMGS_PER_CORE : (c + 1) * IMGS_PER_CORE]}
        for c in range(N_CORES)
    ]


def _combine(results, label):
    y = np.asarray(label, dtype=np.float64).reshape(B)
    ln2 = np.log(2.0)
    num = 0.0
    cnt = 0.0
    for c in range(N_CORES):
        stats = np.asarray(results[c]["stats"], dtype=np.float64)
        sxm_cols = stats[:, 0, :].sum(axis=0)  # [IMGS_PER_CORE + 1]
        ssp_cols = stats[:, 1, :].sum(axis=0)
        # Fold the split last image's two half-chunk columns together.
        sxm_i = np.concatenate([sxm_cols[:-2], [sxm_cols[-2] + sxm_cols[-1]]])
        ssp_i = np.concatenate([ssp_cols[:-2], [ssp_cols[-2] + ssp_cols[-1]]])
        c_core = np.asarray(results[c]["cnt"], dtype=np.float64).sum()
        a_sum = ssp_i.sum() - (IMGS_PER_CORE * N_PER_IMG - c_core) * ln2
        y_i = y[c * IMGS_PER_CORE : (c + 1) * IMGS_PER_CORE]
        num += a_sum - (y_i * sxm_i).sum()
        cnt += c_core
    return np.float32(num / max(cnt, 1.0))


def kernel(cancer_logits, label, prostate_mask, needle_mask):
    nc = _get_nc()
    in_maps = _make_in_maps(cancer_logits, prostate_mask, needle_mask)
    res = run_bass_kernel_spmd(nc, in_maps, core_ids=list(range(N_CORES)))
    return _combine(res.results, label)



# revision 2
# speedup vs baseline: 1.1113x; 1.1113x over previous
"""Masked-BCE valid-region loss on 8 Trainium2 NeuronCores.

Inputs (full): cancer_logits [32,1,512,512] f32, label [32] f32,
prostate_mask [32,1,512,512] f32, needle_mask [32,1,512,512] f32.
Output: scalar f32 loss.

Sharding: data-parallel over batch — 4 images per core. Host packs the
three big tensors into one [IMG, 128, 3, 2048] bf16 input per core
(bf16 halves HBM traffic; the 2e-2 harness tolerance dwarfs the bf16
rounding effect on both numerator and count). Per image the device
computes partial sums; the host combines them:

    bce = softplus(x) - x*y            (y constant per image)
    sum(bce*m) = sum_masked softplus(x) - y * sum(x*m)
    softplus(x*m) = softplus(x) where m==1, ln(2) where m==0
 => sum_masked softplus(x) = sum softplus(x*m) - (N - count)*ln(2)

Device pipeline per chunk (all DVE ops are scalar_tensor_tensor, which
supports the 4x two-pump DVE mode for packed bf16 SBUF operands; plain
tensor_scalar cannot carry an accumulator — the BIR verifier rejects
it — and tensor_tensor only supports 2x):

    g   = (p > 0.5) * n      # g > 0.5  <=>  (p > 0.5) & (n > 0.5)
    cnt = (g > 0.5) * ones   # accum -> mask count
    xm  = (g > 0.5) * x      # accum -> sum(x*m)
    et  = exp(xm)            # ACT
    ssp = ln(et + 1)         # ACT, accum -> sum(softplus(x*m))

The count rides the DVE accumulator (per-chunk column), so TensorE does
nothing and PSUM is unused. The last image is processed as four
quarter-chunks to shorten the serial tail after its DMA lands.
"""

import sys

for _p in ("/opt/trn_rl_repo", "/root/.axon_site/_ro/trn_rl_repo"):
    if _p not in sys.path:
        sys.path.append(_p)

import ml_dtypes
import numpy as np

import concourse.bacc as bacc
import concourse.tile as tile
from concourse import mybir
from concourse.bass_utils import run_bass_kernel_spmd

B, H, W = 32, 512, 512
N_CORES = 8
IMGS_PER_CORE = B // N_CORES  # 4
P = 128
FD = (H * W) // P  # 2048 free-dim elements per partition per image
N_PER_IMG = H * W  # 262144
QF = FD // 4  # quarter-image free dim
N_COLS = IMGS_PER_CORE + 3  # 3 full-image chunks + 4 quarter chunks

_nc_cache = None


def _patch_act_tables():
    """Steer every activation to `natural_log_exp_and_others` (it holds
    exp, ln, copy, identity) by blanking the other sets' function lists.
    The per-activation table picker takes the first set containing the
    function, so without this Exp->set0 / Ln->natural_log alternate and
    bacc emits a ~1.3us ACT_TABLE_LOAD before nearly every ACTIVATE.
    Set positions are preserved, so the emitted act_func_set_id still
    matches act_info.json and the correct table is loaded."""
    import concourse.hw_specs as hw_specs

    if getattr(bacc, "_act_tables_patched", False):
        return
    orig = hw_specs.get_activation_tables

    def patched(module_arch):
        tables = orig(module_arch)
        keep = "natural_log_exp_and_others"
        if keep in tables:
            tables = {
                name: (funcs if name == keep else set())
                for name, funcs in tables.items()
            }
        return tables

    bacc.get_activation_tables = patched
    bacc._act_tables_patched = True


def _build_bass():
    _patch_act_tables()
    f32 = mybir.dt.float32
    bf16 = mybir.dt.bfloat16
    # Bacc (not plain Bass): its finalize() runs generate_event_semaphores,
    # which splits multi-semaphore sync waits into single-wait EventSemaphore
    # instructions — walrus codegen rejects instructions with >1 sync wait.
    nc = bacc.Bacc()
    # Per image: [128 partitions, {logits, prostate, needle}, 2048].
    xpn_d = nc.dram_tensor(
        "xpn", [IMGS_PER_CORE, P, 3, FD], bf16, kind="ExternalInput"
    )
    # stats rows: 0 = per-chunk sum(x*m), 1 = per-chunk mask count,
    # 2 = per-chunk sum(softplus(x*m)); all per-partition. Host folds the
    # last image's four quarter-chunk columns together.
    out_d = nc.dram_tensor("stats", [P, 3, N_COLS], f32, kind="ExternalOutput")

    with tile.TileContext(nc) as tc:
        with (
            tc.tile_pool(name="io", bufs=1) as io_pool,
            tc.tile_pool(name="xm", bufs=4) as xm_pool,
            tc.tile_pool(name="work", bufs=4) as work_pool,
            tc.tile_pool(name="stats", bufs=1) as stats_pool,
        ):
            dve_stats = stats_pool.tile([P, 2, N_COLS], f32)
            out_stats = stats_pool.tile([P, 3, N_COLS], f32)
            sxm = dve_stats[:, 0, :]
            cnt = dve_stats[:, 1, :]
            ssp = out_stats[:, 2, :]
            ones = stats_pool.tile([P, FD], bf16)
            nc.vector.memset(ones, 1.0)

            # chunks: (pn tile, x tile, stats column). Each chunk's load is
            # two DMAs — [p,n] first, then [x] (FIFO order) — so the g/cnt
            # DVE work overlaps the logits transfer; the last image is
            # split into quarters to shorten the serial tail.
            last = IMGS_PER_CORE - 1
            chunk_tiles = []
            for i in range(last):
                tpn = io_pool.tile([P, 2, FD], bf16, tag=f"pn{i}")
                tx = io_pool.tile([P, FD], bf16, tag=f"x{i}")
                nc.sync.dma_start(out=tpn, in_=xpn_d[i][:, 1:3, :])
                nc.sync.dma_start(out=tx, in_=xpn_d[i][:, 0, :])
                chunk_tiles.append((tpn, tx, i))
            for h in range(4):
                sl = slice(h * QF, (h + 1) * QF)
                tpn = io_pool.tile([P, 2, QF], bf16, tag=f"pn{last}h{h}")
                tx = io_pool.tile([P, QF], bf16, tag=f"x{last}h{h}")
                nc.sync.dma_start(out=tpn, in_=xpn_d[last][:, 1:3, sl])
                nc.sync.dma_start(out=tx, in_=xpn_d[last][:, 0, sl])
                chunk_tiles.append((tpn, tx, last + h))

            for tpn, tx, i in chunk_tiles:
                cfd = tpn.shape[2]
                pt = tpn[:, 0, :]
                nt = tpn[:, 1, :]

                # g = (p > 0.5) * n, in place over p.
                nc.vector.scalar_tensor_tensor(
                    out=pt,
                    in0=pt,
                    scalar=0.5,
                    in1=nt,
                    op0=mybir.AluOpType.is_gt,
                    op1=mybir.AluOpType.mult,
                )
                # count: (g > 0.5) * 1 with fused per-partition sum. Needs
                # only the pn transfer, so it runs while the x DMA lands.
                mt = work_pool.tile([P, cfd], bf16, tag="mt")
                nc.vector.scalar_tensor_tensor(
                    out=mt,
                    in0=pt,
                    scalar=0.5,
                    in1=ones[:, :cfd],
                    op0=mybir.AluOpType.is_gt,
                    op1=mybir.AluOpType.mult,
                    accum_out=cnt[:, i : i + 1],
                )
                # xm = (g > 0.5) * x with fused per-partition sum(xm).
                xmt = xm_pool.tile([P, cfd], bf16, tag="xmt")
                nc.vector.scalar_tensor_tensor(
                    out=xmt,
                    in0=pt,
                    scalar=0.5,
                    in1=tx,
                    op0=mybir.AluOpType.is_gt,
                    op1=mybir.AluOpType.mult,
                    accum_out=sxm[:, i : i + 1],
                )
                # softplus(xm) = ln(exp(xm) + 1); |xm| <= ~6 so exp is safe.
                et = work_pool.tile([P, cfd], f32, tag="et")
                nc.scalar.activation(
                    out=et, in_=xmt, func=mybir.ActivationFunctionType.Exp
                )
                nc.scalar.activation(
                    out=et,
                    in_=et,
                    func=mybir.ActivationFunctionType.Ln,
                    bias=1.0,
                    accum_out=ssp[:, i : i + 1],
                )

            # Move DVE-written results into the ACT-owned output tile so the
            # output DMA waits on the ACT semaphore only.
            nc.scalar.activation(
                out=out_stats[:, 0:2, :],
                in_=dve_stats,
                func=mybir.ActivationFunctionType.Copy,
            )
            nc.sync.dma_start(out=out_d[:], in_=out_stats)
    nc.finalize()
    return nc


def _get_nc():
    global _nc_cache
    if _nc_cache is None:
        _nc_cache = _build_bass()
    return _nc_cache


def _make_in_maps(cancer_logits, prostate_mask, needle_mask):
    bf = ml_dtypes.bfloat16
    x = np.asarray(cancer_logits, dtype=np.float32).reshape(B, P, FD).astype(bf)
    p = np.asarray(prostate_mask, dtype=np.float32).reshape(B, P, FD).astype(bf)
    n = np.asarray(needle_mask, dtype=np.float32).reshape(B, P, FD).astype(bf)
    xpn = np.empty((B, P, 3, FD), dtype=bf)
    xpn[:, :, 0, :] = x
    xpn[:, :, 1, :] = p
    xpn[:, :, 2, :] = n
    return [
        {"xpn": xpn[c * IMGS_PER_CORE : (c + 1) * IMGS_PER_CORE]}
        for c in range(N_CORES)
    ]


def _combine(results, label):
    y = np.asarray(label, dtype=np.float64).reshape(B)
    ln2 = np.log(2.0)
    num = 0.0
    cnt = 0.0
    for c in range(N_CORES):
        stats = np.asarray(results[c]["stats"], dtype=np.float64)
        sxm_cols = stats[:, 0, :].sum(axis=0)  # [N_COLS]
        ssp_cols = stats[:, 2, :].sum(axis=0)
        # Fold the split last image's four quarter-chunk columns together.
        nfull = IMGS_PER_CORE - 1
        sxm_i = np.concatenate([sxm_cols[:nfull], [sxm_cols[nfull:].sum()]])
        ssp_i = np.concatenate([ssp_cols[:nfull], [ssp_cols[nfull:].sum()]])
        c_core = stats[:, 1, :].sum()
        a_sum = ssp_i.sum() - (IMGS_PER_CORE * N_PER_IMG - c_core) * ln2
        y_i = y[c * IMGS_PER_CORE : (c + 1) * IMGS_PER_CORE]
        num += a_sum - (y_i * sxm_i).sum()
        cnt += c_core
    return np.float32(num / max(cnt, 1.0))


def kernel(cancer_logits, label, prostate_mask, needle_mask):
    nc = _get_nc()
    in_maps = _make_in_maps(cancer_logits, prostate_mask, needle_mask)
    res = run_bass_kernel_spmd(nc, in_maps, core_ids=list(range(N_CORES)))
    return _combine(res.results, label)


# revision 4
# speedup vs baseline: 1.2391x; 1.1150x over previous
"""Masked-BCE valid-region loss on 8 Trainium2 NeuronCores.

Inputs (full): cancer_logits [32,1,512,512] f32, label [32] f32,
prostate_mask [32,1,512,512] f32, needle_mask [32,1,512,512] f32.
Output: scalar f32 loss.

Sharding: data-parallel over batch — 4 images per core. Host packs the
three big tensors into one [IMG, 128, 3, 2048] bf16 input per core
(bf16 halves HBM traffic; the 2e-2 harness tolerance dwarfs the bf16
rounding effect on both numerator and count). Per image the device
computes partial sums; the host combines them:

    bce = softplus(x) - x*y            (y constant per image)
    sum(bce*m) = sum_masked softplus(x) - y * sum(x*m)
    softplus(x*m) = softplus(x) where m==1, ln(2) where m==0
 => sum_masked softplus(x) = sum softplus(x*m) - (N - count)*ln(2)

Device pipeline per chunk, balanced across four engines:

    pt  = min(p, n)          # DVE tensor_tensor — 2x two-pump at bf16
    m   = (pt > 0.5)         # DVE tensor_scalar — 2x/4x at bf16
    xm  = (pt > 0.5) * x     # DVE scalar_tensor_tensor, accum -> sum(x*m)
    cnt += ones' @ m         # TensorE bf16, accumulated in one PSUM bank
    et  = exp(xm)            # ACT
    ssp = ln(et + 1)         # ACT, accum -> sum(softplus(x*m))

scalar_tensor_tensor never gets the fast DVE modes (its
is_scalar_tensor_tensor form disables them), plain tensor_scalar cannot
carry an accumulator (BIR verifier rejects it), and GPSIMD cannot run
TensorScalarPtr at all — so the count is a cheap 2x/4x tensor_scalar
plus a TensorE ones-matmul reduction on the otherwise idle PE. The
first image is processed as two half-image chunks (compute starts
sooner after its DMA lands) and the last as four quarter-chunks
(shorter serial tail).
"""

import sys

for _p in ("/opt/trn_rl_repo", "/root/.axon_site/_ro/trn_rl_repo"):
    if _p not in sys.path:
        sys.path.append(_p)

import ml_dtypes
import numpy as np

import concourse.bacc as bacc
import concourse.tile as tile
from concourse import mybir
from concourse.bass_utils import run_bass_kernel_spmd

B, H, W = 32, 512, 512
N_CORES = 8
IMGS_PER_CORE = B // N_CORES  # 4
P = 128
FD = (H * W) // P  # 2048 free-dim elements per partition per image
N_PER_IMG = H * W  # 262144
HF = FD // 2
QF = FD // 4
# chunk free-dims: first image in halves, middle images whole, last in quarters
CHUNK_FDS = [HF, HF, FD, FD, QF, QF, QF, QF]
N_COLS = len(CHUNK_FDS)

_nc_cache = None


def _patch_act_tables():
    """Steer every activation to `natural_log_exp_and_others` (it holds
    exp, ln, copy, identity) by blanking the other sets' function lists.
    The per-activation table picker takes the first set containing the
    function, so without this Exp->set0 / Ln->natural_log alternate and
    bacc emits a ~1.3us ACT_TABLE_LOAD before nearly every ACTIVATE.
    Set positions are preserved, so the emitted act_func_set_id still
    matches act_info.json and the correct table is loaded."""
    import concourse.hw_specs as hw_specs

    if getattr(bacc, "_act_tables_patched", False):
        return
    orig = hw_specs.get_activation_tables

    def patched(module_arch):
        tables = orig(module_arch)
        keep = "natural_log_exp_and_others"
        if keep in tables:
            tables = {
                name: (funcs if name == keep else set())
                for name, funcs in tables.items()
            }
        return tables

    bacc.get_activation_tables = patched
    bacc._act_tables_patched = True


def _build_bass():
    _patch_act_tables()
    f32 = mybir.dt.float32
    bf16 = mybir.dt.bfloat16
    # Bacc (not plain Bass): its finalize() runs generate_event_semaphores,
    # which splits multi-semaphore sync waits into single-wait EventSemaphore
    # instructions — walrus codegen rejects instructions with >1 sync wait.
    nc = bacc.Bacc()
    # Per image: [128 partitions, {logits, prostate, needle}, 2048].
    xpn_d = nc.dram_tensor(
        "xpn", [IMGS_PER_CORE, P, 3, FD], bf16, kind="ExternalInput"
    )
    # stats rows: 0 = per-chunk sum(x*m), 1 = per-chunk sum(softplus(x*m)).
    out_d = nc.dram_tensor("stats", [P, 2, N_COLS], f32, kind="ExternalOutput")
    # Mask count, reduced over partitions by TensorE; host sums the 512.
    cnt_d = nc.dram_tensor("cnt", [1, 512], f32, kind="ExternalOutput")

    with tile.TileContext(nc) as tc:
        with (
            tc.tile_pool(name="io", bufs=1) as io_pool,
            tc.tile_pool(name="xm", bufs=4) as xm_pool,
            tc.tile_pool(name="work", bufs=4) as work_pool,
            tc.tile_pool(name="stats", bufs=1) as stats_pool,
            tc.tile_pool(name="psum", bufs=1, space="PSUM") as psum_pool,
        ):
            dve_stats = stats_pool.tile([P, 1, N_COLS], f32)
            out_stats = stats_pool.tile([P, 2, N_COLS], f32)
            sxm = dve_stats[:, 0, :]
            ssp = out_stats[:, 1, :]
            ones = stats_pool.tile([P, 1], bf16)
            nc.vector.memset(ones, 1.0)
            cnt_ps = psum_pool.tile([1, 512], f32)

            # chunks: (pn tile, x tile, stats column). Each chunk's load is
            # two DMAs — [p,n] first, then [x] (FIFO order) — so the min
            # and mask work overlaps the logits transfer.
            chunk_tiles = []
            col = 0
            img = 0
            off = 0
            for cfd in CHUNK_FDS:
                sl = slice(off, off + cfd)
                tpn = io_pool.tile([P, 2, cfd], bf16, tag=f"pn{col}")
                tx = io_pool.tile([P, cfd], bf16, tag=f"x{col}")
                nc.sync.dma_start(out=tpn, in_=xpn_d[img][:, 1:3, sl])
                nc.sync.dma_start(out=tx, in_=xpn_d[img][:, 0, sl])
                chunk_tiles.append((tpn, tx, col))
                col += 1
                off += cfd
                if off == FD:
                    img += 1
                    off = 0

            total_mms = sum(cfd // 512 for cfd in CHUNK_FDS)
            mm_done = 0
            for tpn, tx, i in chunk_tiles:
                cfd = tpn.shape[2]
                pt = tpn[:, 0, :]
                nt = tpn[:, 1, :]

                # pt = min(p, n); (min > 0.5) == (p > 0.5) & (n > 0.5).
                nc.vector.tensor_tensor(
                    out=pt, in0=pt, in1=nt, op=mybir.AluOpType.min
                )
                # mask m = (pt > 0.5) in {0.0, 1.0}; needs only the pn
                # transfer, so it runs while the x DMA lands.
                mt = work_pool.tile([P, cfd], bf16, tag="mt")
                nc.vector.tensor_scalar(
                    out=mt,
                    in0=pt,
                    scalar1=0.5,
                    scalar2=None,
                    op0=mybir.AluOpType.is_gt,
                )
                # count: TensorE reduces m over partitions into PSUM.
                n_sub = cfd // 512
                for c in range(n_sub):
                    nc.tensor.matmul(
                        cnt_ps,
                        ones,
                        mt[:, c * 512 : (c + 1) * 512],
                        start=(mm_done == 0),
                        stop=(mm_done == total_mms - 1),
                    )
                    mm_done += 1
                # xm = (pt > 0.5) * x with fused per-partition sum(xm).
                xmt = xm_pool.tile([P, cfd], bf16, tag="xmt")
                nc.vector.scalar_tensor_tensor(
                    out=xmt,
                    in0=pt,
                    scalar=0.5,
                    in1=tx,
                    op0=mybir.AluOpType.is_gt,
                    op1=mybir.AluOpType.mult,
                    accum_out=sxm[:, i : i + 1],
                )
                # softplus(xm) = ln(exp(xm) + 1); |xm| <= ~6 so exp is safe.
                et = work_pool.tile([P, cfd], f32, tag="et")
                nc.scalar.activation(
                    out=et, in_=xmt, func=mybir.ActivationFunctionType.Exp
                )
                nc.scalar.activation(
                    out=et,
                    in_=et,
                    func=mybir.ActivationFunctionType.Ln,
                    bias=1.0,
                    accum_out=ssp[:, i : i + 1],
                )

            # Move DVE/PE-written results into ACT-owned tiles so each output
            # DMA waits on the ACT semaphore only.
            nc.scalar.activation(
                out=out_stats[:, 0:1, :],
                in_=dve_stats,
                func=mybir.ActivationFunctionType.Copy,
            )
            cnt_sb = stats_pool.tile([1, 512], f32)
            nc.scalar.activation(
                out=cnt_sb, in_=cnt_ps, func=mybir.ActivationFunctionType.Copy
            )
            nc.sync.dma_start(out=out_d[:], in_=out_stats)
            nc.sync.dma_start(out=cnt_d[:], in_=cnt_sb)
    nc.finalize()
    return nc


def _get_nc():
    global _nc_cache
    if _nc_cache is None:
        _nc_cache = _build_bass()
    return _nc_cache


def _make_in_maps(cancer_logits, prostate_mask, needle_mask):
    bf = ml_dtypes.bfloat16
    x = np.asarray(cancer_logits, dtype=np.float32).reshape(B, P, FD).astype(bf)
    p = np.asarray(prostate_mask, dtype=np.float32).reshape(B, P, FD).astype(bf)
    n = np.asarray(needle_mask, dtype=np.float32).reshape(B, P, FD).astype(bf)
    xpn = np.empty((B, P, 3, FD), dtype=bf)
    xpn[:, :, 0, :] = x
    xpn[:, :, 1, :] = p
    xpn[:, :, 2, :] = n
    return [
        {"xpn": xpn[c * IMGS_PER_CORE : (c + 1) * IMGS_PER_CORE]}
        for c in range(N_CORES)
    ]


# map stats column -> image index (for per-image sum(x*m) folding)
_COL_IMG = []
_img = 0
_off = 0
for _cfd in CHUNK_FDS:
    _COL_IMG.append(_img)
    _off += _cfd
    if _off == FD:
        _img += 1
        _off = 0


def _combine(results, label):
    y = np.asarray(label, dtype=np.float64).reshape(B)
    ln2 = np.log(2.0)
    num = 0.0
    cnt = 0.0
    col_img = np.asarray(_COL_IMG)
    for c in range(N_CORES):
        stats = np.asarray(results[c]["stats"], dtype=np.float64)
        sxm_cols = stats[:, 0, :].sum(axis=0)  # [N_COLS]
        sxm_i = np.zeros(IMGS_PER_CORE)
        np.add.at(sxm_i, col_img, sxm_cols)
        c_core = np.asarray(results[c]["cnt"], dtype=np.float64).sum()
        ssp_all = stats[:, 1, :].sum()
        a_sum = ssp_all - (IMGS_PER_CORE * N_PER_IMG - c_core) * ln2
        y_i = y[c * IMGS_PER_CORE : (c + 1) * IMGS_PER_CORE]
        num += a_sum - (y_i * sxm_i).sum()
        cnt += c_core
    return np.float32(num / max(cnt, 1.0))


def kernel(cancer_logits, label, prostate_mask, needle_mask):
    nc = _get_nc()
    in_maps = _make_in_maps(cancer_logits, prostate_mask, needle_mask)
    res = run_bass_kernel_spmd(nc, in_maps, core_ids=list(range(N_CORES)))
    return _combine(res.results, label)
